# revision 20
# baseline (speedup 1.0000x reference)
"""NodeConv kernel for 8 Trainium2 NeuronCores.

Reference computes, for adj [B,1,N,N], node [B,nin,N], Wi/Wj [nout,nin]:
    x  = node[:, :, None, :] * adj          # [B,nin,N,N]
    yi = einsum('oc,bcij->boij', Wi, x)
    yj = einsum('oc,bcij->boij', Wj, x)
    out = I * yi + (1-I) * yj

Because adj[b,i,j] does not depend on the contraction channel c, the
contraction factors out:
    off-diag: out[b,o,i,j] = adj[b,i,j] * (Wj @ node[b])[o,j]
    diag:     out[b,o,j,j] = adj[b,j,j] * (Wi @ node[b])[o,j]

Sharding: core c handles batch b=c//2, row half h=c%2 (128 rows). Odd
halves get their columns rolled by -128 on the host so the diagonal of
local row l sits at local column l on every core -> one SPMD program;
the host rolls the output back while gathering.

v2 changes vs the 72us baseline (all justified by the cost model in
bass_rust_src/instruction_cost_v2.rs + hw_specs.py):
  - OUTPUT IS BF16 (host upconverts to f32 while gathering): halves the
    HBM store traffic 16 MiB -> 8 MiB per core.  Tolerance is 2e-2;
    measured end-to-end max rel err of this path is ~7e-3.
  - adj broadcast matmuls use FP8E4 DoubleRow perf mode: adj is split
    host-side into 4 scaled e4m3 terms (reconstruction error ~4e-6
    absolute) packed as DoubleRow (k, pair) operands with K=2, and the
    stationary selector carries the 2^-4/2^-8 descale factors.  0.5
    PE cycles per output column instead of 1.0, and the stationary is
    the same tiny [2,256] tile for every matmul.
  - the per-chunk multiply out = bcast(adj) * u_rep is split between
    two pipelines to beat DVE's 1-elem/lane/cycle f32 limit:
      A-chunks: DVE tensor_mul straight from PSUM (f32) -> bf16 SBUF.
      B-chunks: ScalarE copies PSUM f32 -> bf16 SBUF staging, then DVE
        multiplies all-bf16 (2x_1P mode: 2 elem/lane/cycle).
    This puts ~22us on DVE and ~22us on ScalarE instead of 36us on DVE.
  - diagonal patches run on the otherwise-idle GpSimd engine.
  - store DMAs are issued from the Sync and GpSimd queues (ScalarE is
    busy with copies now), groups front/tail-loaded small.
"""

import os

import numpy as np

NCORES = 8
B, N, NIN, NOUT = 4, 256, 128, 128
RPC = 128          # rows per core
RCH = int(os.environ.get("NODECONV_RCH", "4"))   # rows per chunk
CH = RPC // RCH    # chunks per core
FREE = RCH * N     # free elems per chunk
PSUM_BUFS = int(os.environ.get("NODECONV_PSUM_BUFS", "4" if RCH <= 4 else "2"))

# G-chunks: GpSimd partition_broadcast of host-rounded bf16 adj + DVE 2x_1P
# multiply — no PE/PSUM/ScalarE involvement.  Placed at the edges so the
# first stores fire early and the tail chunks are compute-ready early.
_GSET = {
    int(x)
    for x in os.environ.get("NODECONV_GSET", "").split(",")
    if x != ""
}
# A-chunks: multiplied directly from PSUM on DVE (1x); remaining (B) chunks
# go through a ScalarE bf16 staging copy + DVE 2x_1P multiply
_ASET = {
    int(x)
    for x in os.environ.get(
        "NODECONV_ASET",
        "0,4,8,11,15,18,22,25,29" if RCH == 4 else "0,5,10,15",
    ).split(",")
    if x != ""
}
_G = [
    int(x)
    for x in os.environ.get(
        "NODECONV_GROUPS",
        "4,4,4,4,4,4,4,2,2" if RCH == 4 else "2,2,2,2,2,2,2,1,1",
    ).split(",")
]
assert sum(_G) == CH
STAGE_BUFS = int(os.environ.get("NODECONV_STAGE_BUFS", "6"))
OUT_BUFS = int(os.environ.get("NODECONV_OUT_BUFS", "6"))
PATCH_ENG = os.environ.get("NODECONV_PATCH", "gp")  # gp | scalar | vector

KP = 2 * CH        # fp8 operand partitions (2 per chunk)

_cached = {}

last_results = None  # BassKernelResults of the most recent kernel() call


def _build_nc():
    key = (tuple(sorted(_ASET)), tuple(_G), STAGE_BUFS, OUT_BUFS, PATCH_ENG)
    if key in _cached:
        return _cached[key]

    from contextlib import ExitStack

    import concourse.tile as tile
    from concourse import bacc, mybir

    f32 = mybir.dt.float32
    bf16 = mybir.dt.bfloat16
    fp8 = mybir.dt.float8e4
    DR = mybir.MatmulPerfMode.DoubleRow

    nc = bacc.Bacc(
        "TRN2", target_bir_lowering=False, debug=False, num_devices=NCORES
    )

    # pk8: [32, 2*FREE + CH*2*NOUT] fp8 — DoubleRow adj terms (partition 2p
    # holds (t0,t1) of chunk p at [i*FREE + col], partition 2p+1 (t2,t3)),
    # then the DoubleRow stationary selector: chunk p's [32, 2, NOUT] block
    # is nonzero only on partitions 2p (1, 2^-4) and 2p+1 (2^-8, 2^-8) —
    # the descale factors of the adj terms
    pk8 = nc.dram_tensor(
        "pk8", [KP, 2 * FREE + CH * 2 * NOUT], fp8, kind="ExternalInput"
    ).ap()
    # ckf: [128, 640] bf16 — node_r | WiT | WjT | nodeD, where nodeD is the
    # diagonal node columns prescaled by adj's diagonal on the host, so
    # Wi @ nodeD directly yields the diagonal patch values dv.  bf16 keeps
    # the u/dv matmuls single-pass (fp32 PE matmuls run as 2 half-speed
    # passes) and halves the critical first input load.
    ckf = nc.dram_tensor(
        "ckf", [NIN, N + 2 * NOUT + RPC], bf16, kind="ExternalInput"
    ).ap()
    # af: chunk p's eight adj rows flattened on partition 0 (bf16) — the
    # source for GpSimd partition_broadcast on G-chunks
    af = nc.dram_tensor("af", [1, RPC * N], bf16, kind="ExternalInput").ap()
    out = nc.dram_tensor("out", [NOUT, RPC * N], bf16, kind="ExternalOutput").ap()

    with tile.TileContext(nc) as tc, ExitStack() as ctx:
        const = ctx.enter_context(tc.tile_pool(name="const", bufs=1))
        psum = ctx.enter_context(tc.tile_pool(name="psum", bufs=PSUM_BUFS, space="PSUM"))
        outp = ctx.enter_context(tc.tile_pool(name="outp", bufs=OUT_BUFS))
        stage = ctx.enter_context(tc.tile_pool(name="stage", bufs=STAGE_BUFS))
        bcp = ctx.enter_context(tc.tile_pool(name="bcp", bufs=len(_GSET) or 1))

        # Both loads on the sync queue (it issues earliest), ckf first: it is
        # smaller and heads the u-chain that every multiply depends on.
        ckf_sb = const.tile([NIN, N + 2 * NOUT + RPC], bf16)
        nc.sync.dma_start(out=ckf_sb[:], in_=ckf)
        pk8_sb = const.tile([KP, 2 * FREE + CH * 2 * NOUT], fp8)
        nc.sync.dma_start(out=pk8_sb[:], in_=pk8)
        af_sb = const.tile([1, RPC * N], bf16)
        nc.gpsimd.dma_start(out=af_sb[:], in_=af)

        node_sb = ckf_sb[:, 0:N]
        wit_sb = ckf_sb[:, N : N + NOUT]
        wjt_sb = ckf_sb[:, N + NOUT : N + 2 * NOUT]
        noded_sb = ckf_sb[:, N + 2 * NOUT : N + 2 * NOUT + RPC]
        sel_v = pk8_sb[:, 2 * FREE :].rearrange("k (p i o) -> k p i o", p=CH, i=2)
        pk_v = pk8_sb[:, 0 : 2 * FREE].rearrange("k (i f) -> k i f", i=2)

        # u = Wj @ node_r -> [nout, N], and (later) dv = Wi @ nodeD.  Both
        # live in one psum tile so the chunk psum tiles keep alternating
        # between the pool's two buffers.
        ps_uv = psum.tile([NOUT, N + RPC], f32, tag="mm")
        nc.tensor.matmul(
            ps_uv[:, 0:N], lhsT=wjt_sb, rhs=node_sb, start=True, stop=True
        )
        u_bf = const.tile([NOUT, N], bf16)
        nc.scalar.copy(u_bf[:], ps_uv[:, 0:N])
        # dv = Wi @ nodeD — must be written before the first patch reads it
        dv_bf = const.tile([NOUT, RPC], bf16)
        nc.tensor.matmul(
            ps_uv[:, N : N + RPC], lhsT=wit_sb, rhs=noded_sb, start=True, stop=True
        )
        nc.scalar.copy(dv_bf[:], ps_uv[:, N : N + RPC])

        # u replicated RCH times along the free dim via a stride-0 view
        u16_rep = u_bf[:].unsqueeze(1).broadcast_to([NOUT, RCH, N])

        patch_eng = {
            "gp": nc.gpsimd,
            "scalar": nc.scalar,
            "vector": nc.vector,
        }[PATCH_ENG]

        p = 0
        for gi, gsz in enumerate(_G):
            o_sb = outp.tile([NOUT, gsz * FREE], bf16, tag="osb")
            p0 = p
            for g in range(gsz):
                o_view = o_sb[:, g * FREE : (g + 1) * FREE].rearrange(
                    "p (k j) -> p k j", k=RCH
                )
                if p in _GSET:
                    bc = bcp.tile([NOUT, FREE], bf16, tag="bc")
                    nc.gpsimd.partition_broadcast(
                        bc[:], af_sb[:, FREE * p : FREE * (p + 1)]
                    )
                    nc.vector.tensor_mul(
                        o_view, bc[:].rearrange("p (k j) -> p k j", k=RCH), u16_rep
                    )
                    peng = nc.vector
                else:
                    ps_b = psum.tile([NOUT, FREE], f32, tag="mm")
                    lhs_c = sel_v[:, p]
                    for q in range(FREE // 512):
                        nc.tensor.matmul(
                            ps_b[:, 512 * q : 512 * (q + 1)],
                            lhsT=lhs_c,
                            rhs=pk_v[:, :, 512 * q : 512 * (q + 1)],
                            start=True,
                            stop=True,
                            perf_mode=DR,
                        )
                    if p in _ASET:
                        nc.vector.tensor_mul(
                            o_view,
                            ps_b[:].rearrange("p (k j) -> p k j", k=RCH),
                            u16_rep,
                        )
                    else:
                        st = stage.tile([NOUT, FREE], bf16, tag="st")
                        nc.scalar.copy(st[:], ps_b[:])
                        nc.vector.tensor_mul(
                            o_view,
                            st[:].rearrange("p (k j) -> p k j", k=RCH),
                            u16_rep,
                        )
                    peng = patch_eng
                # diagonal of local row l=8p+k sits at free offset 8p + k*257
                peng.tensor_scalar_mul(
                    o_sb[
                        :,
                        g * FREE + RCH * p : g * FREE
                        + RCH * p
                        + (RCH - 1) * (N + 1)
                        + 1 : N + 1,
                    ],
                    dv_bf[:, RCH * p : RCH * (p + 1)],
                    1.0,
                )
                p += 1
            eng = nc.sync if gi % 2 == 0 else nc.gpsimd
            eng.dma_start(out=out[:, FREE * p0 : FREE * p], in_=o_sb[:])

    nc.compile()
    _cached[key] = nc
    return nc


def _split_fp8_terms(x):
    """Split fp32 array (values in [0,1)) into 4 e4m3 terms with scales
    (1, 2^4, 2^8, 2^8) whose descaled f32 sum reconstructs x to ~4e-6."""
    import ml_dtypes

    f8 = ml_dtypes.float8_e4m3
    t0 = x.astype(f8)
    r = x - t0.astype(np.float32)
    t1 = (r * 16.0).astype(f8)
    r = r - t1.astype(np.float32) / 16.0
    t2 = (r * 256.0).astype(f8)
    r = r - t2.astype(np.float32) / 256.0
    t3 = (r * 256.0).astype(f8)
    return t0, t1, t2, t3


def _in_maps(adj, node, Wi, Wj):
    import ml_dtypes

    f8 = ml_dtypes.float8_e4m3
    sel = np.zeros((KP, CH, 2, NOUT), f8)
    for p in range(CH):
        sel[2 * p, p, 0, :] = 1.0
        sel[2 * p, p, 1, :] = 2.0**-4
        sel[2 * p + 1, p, :, :] = 2.0**-8
    sel = sel.reshape(KP, CH * 2 * NOUT)
    bf = ml_dtypes.bfloat16
    ckf = np.empty((NIN, N + 2 * NOUT + RPC), bf)
    ckf[:, N : N + NOUT] = Wi.T
    ckf[:, N + NOUT : N + 2 * NOUT] = Wj.T
    bf = ml_dtypes.bfloat16
    maps = []
    for c in range(NCORES):
        b, h = divmod(c, 2)
        r0 = RPC * h
        a = adj[b, 0, r0 : r0 + RPC, :]
        if h:
            ar = np.roll(a, -r0, axis=1)
            noder = np.roll(node[b], -r0, axis=1)
        else:
            ar = a
            noder = node[b]
        t0, t1, t2, t3 = _split_fp8_terms(ar.reshape(CH, FREE))
        pk8 = np.empty((KP, 2 * FREE + CH * 2 * NOUT), f8)
        pk8[0::2, 0:FREE] = t0
        pk8[0::2, FREE : 2 * FREE] = t1
        pk8[1::2, 0:FREE] = t2
        pk8[1::2, FREE : 2 * FREE] = t3
        pk8[:, 2 * FREE :] = sel
        m_ckf = ckf.copy()
        m_ckf[:, 0:N] = noder
        adiag = a[np.arange(RPC), r0 + np.arange(RPC)]
        m_ckf[:, N + 2 * NOUT :] = noder[:, 0:RPC] * adiag[None, :]
        af = ar.reshape(1, RPC * N).astype(bf)
        maps.append({"pk8": pk8, "ckf": m_ckf, "af": af})
    return maps


def kernel(**inputs):
    global last_results
    adj = np.asarray(inputs["adj"], dtype=np.float32)
    node = np.asarray(inputs["node"], dtype=np.float32)
    Wi = np.asarray(inputs["Wi"], dtype=np.float32)
    Wj = np.asarray(inputs["Wj"], dtype=np.float32)

    from concourse.bass_utils import run_bass_kernel_spmd

    nc = _build_nc()
    res = run_bass_kernel_spmd(nc, _in_maps(adj, node, Wi, Wj), list(range(NCORES)))
    last_results = res

    out = np.empty((B, NOUT, N, N), np.float32)
    for c in range(NCORES):
        b, h = divmod(c, 2)
        co = res.results[c]["out"].astype(np.float32).reshape(NOUT, RPC, N)
        if h:
            co = np.roll(co, RPC * h, axis=2)
        out[b, :, RPC * h : RPC * (h + 1), :] = co
    return out


# revision 21
# speedup vs baseline: 1.0819x; 1.0819x over previous
"""NodeConv kernel for 8 Trainium2 NeuronCores.

Reference computes, for adj [B,1,N,N], node [B,nin,N], Wi/Wj [nout,nin]:
    x  = node[:, :, None, :] * adj          # [B,nin,N,N]
    yi = einsum('oc,bcij->boij', Wi, x)
    yj = einsum('oc,bcij->boij', Wj, x)
    out = I * yi + (1-I) * yj

Because adj[b,i,j] does not depend on the contraction channel c, the
contraction factors out:
    off-diag: out[b,o,i,j] = adj[b,i,j] * (Wj @ node[b])[o,j]
    diag:     out[b,o,j,j] = adj[b,j,j] * (Wi @ node[b])[o,j]

Sharding: core c handles batch b=c//2, row half h=c%2 (128 rows). Odd
halves get their columns rolled by -128 on the host so the diagonal of
local row l sits at local column l on every core -> one SPMD program;
the host rolls the output back while gathering.

v2 changes vs the 72us baseline (all justified by the cost model in
bass_rust_src/instruction_cost_v2.rs + hw_specs.py):
  - OUTPUT IS BF16 (host upconverts to f32 while gathering): halves the
    HBM store traffic 16 MiB -> 8 MiB per core.  Tolerance is 2e-2;
    measured end-to-end max rel err of this path is ~7e-3.
  - adj broadcast matmuls use FP8E4 DoubleRow perf mode: adj is split
    host-side into 4 scaled e4m3 terms (reconstruction error ~4e-6
    absolute) packed as DoubleRow (k, pair) operands with K=2, and the
    stationary selector carries the 2^-4/2^-8 descale factors.  0.5
    PE cycles per output column instead of 1.0, and the stationary is
    the same tiny [2,256] tile for every matmul.
  - the per-chunk multiply out = bcast(adj) * u_rep is split between
    two pipelines to beat DVE's 1-elem/lane/cycle f32 limit:
      A-chunks: DVE tensor_mul straight from PSUM (f32) -> bf16 SBUF.
      B-chunks: ScalarE copies PSUM f32 -> bf16 SBUF staging, then DVE
        multiplies all-bf16 (2x_1P mode: 2 elem/lane/cycle).
    This puts ~22us on DVE and ~22us on ScalarE instead of 36us on DVE.
  - diagonal patches run on the otherwise-idle GpSimd engine.
  - store DMAs are issued from the Sync and GpSimd queues (ScalarE is
    busy with copies now), groups front/tail-loaded small.
"""

import os

import numpy as np

NCORES = 8
B, N, NIN, NOUT = 4, 256, 128, 128
RPC = 128          # rows per core
RCH = int(os.environ.get("NODECONV_RCH", "4"))   # rows per chunk
CH = RPC // RCH    # chunks per core
FREE = RCH * N     # free elems per chunk
PSUM_BUFS = int(os.environ.get("NODECONV_PSUM_BUFS", "4" if RCH <= 4 else "2"))

# G-chunks: GpSimd partition_broadcast of host-rounded bf16 adj + DVE 2x_1P
# multiply — no PE/PSUM/ScalarE involvement.  Placed at the edges so the
# first stores fire early and the tail chunks are compute-ready early.
_GSET = {
    int(x)
    for x in os.environ.get("NODECONV_GSET", "").split(",")
    if x != ""
}
# A-chunks: multiplied directly from PSUM on DVE (1x); remaining (B) chunks
# go through a ScalarE bf16 staging copy + DVE 2x_1P multiply
_ASET = {
    int(x)
    for x in os.environ.get(
        "NODECONV_ASET",
        "0,4,8,12,16,20,24,30,31" if RCH == 4 else "0,5,10,15",
    ).split(",")
    if x != ""
}
_G = [
    int(x)
    for x in os.environ.get(
        "NODECONV_GROUPS",
        "1,1,2,2,4,4,4,4,4,4,1,1" if RCH == 4 else "2,2,2,2,2,2,2,1,1",
    ).split(",")
]
assert sum(_G) == CH
STAGE_BUFS = int(os.environ.get("NODECONV_STAGE_BUFS", "6"))
OUT_BUFS = int(os.environ.get("NODECONV_OUT_BUFS", "6"))
PATCH_ENG = os.environ.get("NODECONV_PATCH", "gp")  # gp | scalar | vector

KP = 2 * CH        # fp8 operand partitions (2 per chunk)

_cached = {}

last_results = None  # BassKernelResults of the most recent kernel() call


def _build_nc():
    key = (tuple(sorted(_ASET)), tuple(_G), STAGE_BUFS, OUT_BUFS, PATCH_ENG)
    if key in _cached:
        return _cached[key]

    from contextlib import ExitStack

    import concourse.tile as tile
    from concourse import bacc, mybir

    f32 = mybir.dt.float32
    bf16 = mybir.dt.bfloat16
    fp8 = mybir.dt.float8e4
    DR = mybir.MatmulPerfMode.DoubleRow

    nc = bacc.Bacc(
        "TRN2", target_bir_lowering=False, debug=False, num_devices=NCORES
    )

    # pk8: [32, 2*FREE + CH*2*NOUT] fp8 — DoubleRow adj terms (partition 2p
    # holds (t0,t1) of chunk p at [i*FREE + col], partition 2p+1 (t2,t3)),
    # then the DoubleRow stationary selector: chunk p's [32, 2, NOUT] block
    # is nonzero only on partitions 2p (1, 2^-4) and 2p+1 (2^-8, 2^-8) —
    # the descale factors of the adj terms
    pk8 = nc.dram_tensor(
        "pk8", [KP, 2 * FREE + CH * 2 * NOUT], fp8, kind="ExternalInput"
    ).ap()
    # ckf: [128, 640] bf16 — node_r | WiT | WjT | nodeD, where nodeD is the
    # diagonal node columns prescaled by adj's diagonal on the host, so
    # Wi @ nodeD directly yields the diagonal patch values dv.  bf16 keeps
    # the u/dv matmuls single-pass (fp32 PE matmuls run as 2 half-speed
    # passes) and halves the critical first input load.
    ckf = nc.dram_tensor(
        "ckf", [NIN, N + 2 * NOUT + RPC], bf16, kind="ExternalInput"
    ).ap()
    # af: chunk p's eight adj rows flattened on partition 0 (bf16) — the
    # source for GpSimd partition_broadcast on G-chunks
    af = nc.dram_tensor("af", [1, RPC * N], bf16, kind="ExternalInput").ap()
    out = nc.dram_tensor("out", [NOUT, RPC * N], bf16, kind="ExternalOutput").ap()

    with tile.TileContext(nc) as tc, ExitStack() as ctx:
        const = ctx.enter_context(tc.tile_pool(name="const", bufs=1))
        psum = ctx.enter_context(tc.tile_pool(name="psum", bufs=PSUM_BUFS, space="PSUM"))
        outp = ctx.enter_context(tc.tile_pool(name="outp", bufs=OUT_BUFS))
        stage = ctx.enter_context(tc.tile_pool(name="stage", bufs=STAGE_BUFS))
        bcp = ctx.enter_context(tc.tile_pool(name="bcp", bufs=len(_GSET) or 1))

        # Both loads on the sync queue (it issues earliest), ckf first: it is
        # smaller and heads the u-chain that every multiply depends on.
        ckf_sb = const.tile([NIN, N + 2 * NOUT + RPC], bf16)
        nc.sync.dma_start(out=ckf_sb[:], in_=ckf)
        pk8_sb = const.tile([KP, 2 * FREE + CH * 2 * NOUT], fp8)
        nc.sync.dma_start(out=pk8_sb[:], in_=pk8)
        af_sb = const.tile([1, RPC * N], bf16)
        nc.gpsimd.dma_start(out=af_sb[:], in_=af)

        node_sb = ckf_sb[:, 0:N]
        wit_sb = ckf_sb[:, N : N + NOUT]
        wjt_sb = ckf_sb[:, N + NOUT : N + 2 * NOUT]
        noded_sb = ckf_sb[:, N + 2 * NOUT : N + 2 * NOUT + RPC]
        sel_v = pk8_sb[:, 2 * FREE :].rearrange("k (p i o) -> k p i o", p=CH, i=2)
        pk_v = pk8_sb[:, 0 : 2 * FREE].rearrange("k (i f) -> k i f", i=2)

        # u = Wj @ node_r -> [nout, N], and (later) dv = Wi @ nodeD.  Both
        # live in one psum tile so the chunk psum tiles keep alternating
        # between the pool's two buffers.
        ps_uv = psum.tile([NOUT, N + RPC], f32, tag="mm")
        nc.tensor.matmul(
            ps_uv[:, 0:N], lhsT=wjt_sb, rhs=node_sb, start=True, stop=True
        )
        u_bf = const.tile([NOUT, N], bf16)
        nc.scalar.copy(u_bf[:], ps_uv[:, 0:N])
        # dv = Wi @ nodeD — must be written before the first patch reads it
        dv_bf = const.tile([NOUT, RPC], bf16)
        nc.tensor.matmul(
            ps_uv[:, N : N + RPC], lhsT=wit_sb, rhs=noded_sb, start=True, stop=True
        )
        nc.scalar.copy(dv_bf[:], ps_uv[:, N : N + RPC])

        # u replicated RCH times along the free dim via a stride-0 view
        u16_rep = u_bf[:].unsqueeze(1).broadcast_to([NOUT, RCH, N])

        patch_eng = {
            "gp": nc.gpsimd,
            "scalar": nc.scalar,
            "vector": nc.vector,
        }[PATCH_ENG]

        p = 0
        for gi, gsz in enumerate(_G):
            o_sb = outp.tile([NOUT, gsz * FREE], bf16, tag="osb")
            p0 = p
            for g in range(gsz):
                o_view = o_sb[:, g * FREE : (g + 1) * FREE].rearrange(
                    "p (k j) -> p k j", k=RCH
                )
                if p in _GSET:
                    bc = bcp.tile([NOUT, FREE], bf16, tag="bc")
                    nc.gpsimd.partition_broadcast(
                        bc[:], af_sb[:, FREE * p : FREE * (p + 1)]
                    )
                    nc.vector.tensor_mul(
                        o_view, bc[:].rearrange("p (k j) -> p k j", k=RCH), u16_rep
                    )
                    peng = nc.vector
                else:
                    ps_b = psum.tile([NOUT, FREE], f32, tag="mm")
                    lhs_c = sel_v[:, p]
                    for q in range(FREE // 512):
                        nc.tensor.matmul(
                            ps_b[:, 512 * q : 512 * (q + 1)],
                            lhsT=lhs_c,
                            rhs=pk_v[:, :, 512 * q : 512 * (q + 1)],
                            start=True,
                            stop=True,
                            perf_mode=DR,
                        )
                    if p in _ASET:
                        nc.vector.tensor_mul(
                            o_view,
                            ps_b[:].rearrange("p (k j) -> p k j", k=RCH),
                            u16_rep,
                        )
                    else:
                        st = stage.tile([NOUT, FREE], bf16, tag="st")
                        nc.scalar.copy(st[:], ps_b[:])
                        nc.vector.tensor_mul(
                            o_view,
                            st[:].rearrange("p (k j) -> p k j", k=RCH),
                            u16_rep,
                        )
                    peng = patch_eng
                # diagonal of local row l=8p+k sits at free offset 8p + k*257
                peng.tensor_scalar_mul(
                    o_sb[
                        :,
                        g * FREE + RCH * p : g * FREE
                        + RCH * p
                        + (RCH - 1) * (N + 1)
                        + 1 : N + 1,
                    ],
                    dv_bf[:, RCH * p : RCH * (p + 1)],
                    1.0,
                )
                p += 1
            eng = nc.sync if gi % 2 == 0 else nc.gpsimd
            eng.dma_start(out=out[:, FREE * p0 : FREE * p], in_=o_sb[:])

    nc.compile()
    _cached[key] = nc
    return nc


def _split_fp8_terms(x):
    """Split fp32 array (values in [0,1)) into 4 e4m3 terms with scales
    (1, 2^4, 2^8, 2^8) whose descaled f32 sum reconstructs x to ~4e-6."""
    import ml_dtypes

    f8 = ml_dtypes.float8_e4m3
    t0 = x.astype(f8)
    r = x - t0.astype(np.float32)
    t1 = (r * 16.0).astype(f8)
    r = r - t1.astype(np.float32) / 16.0
    t2 = (r * 256.0).astype(f8)
    r = r - t2.astype(np.float32) / 256.0
    t3 = (r * 256.0).astype(f8)
    return t0, t1, t2, t3


def _in_maps(adj, node, Wi, Wj):
    import ml_dtypes

    f8 = ml_dtypes.float8_e4m3
    sel = np.zeros((KP, CH, 2, NOUT), f8)
    for p in range(CH):
        sel[2 * p, p, 0, :] = 1.0
        sel[2 * p, p, 1, :] = 2.0**-4
        sel[2 * p + 1, p, :, :] = 2.0**-8
    sel = sel.reshape(KP, CH * 2 * NOUT)
    bf = ml_dtypes.bfloat16
    ckf = np.empty((NIN, N + 2 * NOUT + RPC), bf)
    ckf[:, N : N + NOUT] = Wi.T
    ckf[:, N + NOUT : N + 2 * NOUT] = Wj.T
    bf = ml_dtypes.bfloat16
    maps = []
    for c in range(NCORES):
        b, h = divmod(c, 2)
        r0 = RPC * h
        a = adj[b, 0, r0 : r0 + RPC, :]
        if h:
            ar = np.roll(a, -r0, axis=1)
            noder = np.roll(node[b], -r0, axis=1)
        else:
            ar = a
            noder = node[b]
        t0, t1, t2, t3 = _split_fp8_terms(ar.reshape(CH, FREE))
        pk8 = np.empty((KP, 2 * FREE + CH * 2 * NOUT), f8)
        pk8[0::2, 0:FREE] = t0
        pk8[0::2, FREE : 2 * FREE] = t1
        pk8[1::2, 0:FREE] = t2
        pk8[1::2, FREE : 2 * FREE] = t3
        pk8[:, 2 * FREE :] = sel
        m_ckf = ckf.copy()
        m_ckf[:, 0:N] = noder
        adiag = a[np.arange(RPC), r0 + np.arange(RPC)]
        m_ckf[:, N + 2 * NOUT :] = noder[:, 0:RPC] * adiag[None, :]
        af = ar.reshape(1, RPC * N).astype(bf)
        maps.append({"pk8": pk8, "ckf": m_ckf, "af": af})
    return maps


def kernel(**inputs):
    global last_results
    adj = np.asarray(inputs["adj"], dtype=np.float32)
    node = np.asarray(inputs["node"], dtype=np.float32)
    Wi = np.asarray(inputs["Wi"], dtype=np.float32)
    Wj = np.asarray(inputs["Wj"], dtype=np.float32)

    from concourse.bass_utils import run_bass_kernel_spmd

    nc = _build_nc()
    res = run_bass_kernel_spmd(nc, _in_maps(adj, node, Wi, Wj), list(range(NCORES)))
    last_results = res

    out = np.empty((B, NOUT, N, N), np.float32)
    for c in range(NCORES):
        b, h = divmod(c, 2)
        co = res.results[c]["out"].astype(np.float32).reshape(NOUT, RPC, N)
        if h:
            co = np.roll(co, RPC * h, axis=2)
        out[b, :, RPC * h : RPC * (h + 1), :] = co
    return out
